# revision 1
# baseline (speedup 1.0000x reference)
"""Megatron-style TP attention kernel for trn2 (8 NeuronCores).

Problem: LayerNorm -> fused QKV -> causal MHA -> fp16 output projection.
  B=2, S=2048, M=2048, H=16 heads, D=128.

Sharding: DP=2 over batch x TP=4 over heads. Core c handles batch c//4 and
heads 4*(c%4)..4*(c%4)+3. Each core computes its 4 heads' context, all 8
cores AllGather the fp16 context (8-rank collective: the 4-rank grouped
variant runs a slow fold_n=2 ring), and each core then computes a disjoint
512-column slice of the output projection for its batch half — no
all-reduce. The host reassembles the full [B,S,M] output.

On-device layouts are "transposed" (contraction dim on partitions):
  xT [m, s], qT/kT [d, s] per head, v [s, d], ctxT [i, s].
LayerNorm is algebraically folded into the QKV eviction:
  qkv = (x - mu) rstd @ W = rstd*(x@W - mu*colsum(W)), so the PE consumes
raw x and never waits for the stats; mean/rstd are applied on the DVE
during PSUM eviction. Column stats come from ones-matmuls on the PE.
Softmax needs no max-subtraction (scores are tiny; masked lanes get exact
zeros via multiplicative masks after exp); normalization is deferred to
the probs@V eviction. Matmuls use float32r (full PE rate at free>=256);
the output projection uses fp16 operands like the reference.

The AllGather is split in two waves (heads 0-2, head 3) so wave 1 and the
wave-1 output matmuls overlap the tail of attention.
"""

import numpy as np

import concourse.bass as bass
import concourse.mybir as mybir
import concourse.tile as tile
from concourse import bacc
from concourse.bass_utils import run_bass_kernel_spmd

FP32 = mybir.dt.float32
FP32R = mybir.dt.float32r
FP16 = mybir.dt.float16
STT_ADD = mybir.AluOpType.add
STT_MULT = mybir.AluOpType.mult

N_CORES = 8
B, S, M, H = 2, 2048, 2048, 16
D = M // H            # 128
TP = 4                # head groups (tensor parallel)
DP = 2                # batch (data parallel)
HPC = H // TP         # 4 heads per core
NSL = HPC * D         # 512: per-core q/k/v and output column slice
EPS = 1e-5
P = 128
SC = 512              # s-chunk
NCH = S // SC         # 4
MT = M // P           # 16
ST = S // P           # 16
import os
SPLIT_AG = os.environ.get("SPLIT_AG", "1") == "1"
W1H = (HPC - 1) if SPLIT_AG else HPC  # heads in AllGather wave 1

_cached = {}


def build_program():
    nc = bacc.Bacc(
        "TRN2",
        target_bir_lowering=False,
        debug=False,
        num_devices=N_CORES,
        enable_partition_id=True,
    )

    xT = nc.dram_tensor("xT", [M, S], FP32, kind="ExternalInput")
    # q/k weights, host-pretiled: [nt, p, mt*128+n] so each nt-tile DMA is
    # one 8KB-contiguous run per partition
    wqk_t = nc.dram_tensor("wqk_t", [8, P, MT * P], FP32, kind="ExternalInput")
    wv = nc.dram_tensor("wv", [M, NSL], FP32, kind="ExternalInput")
    # negated column sums of the (g-folded) weights, for the mean fold
    wsqk = nc.dram_tensor("wsqk", [P, 8], FP32, kind="ExternalInput")
    wvs = nc.dram_tensor("wvs", [1, NSL], FP32, kind="ExternalInput")
    bqk = nc.dram_tensor("bqk", [P, 8], FP32, kind="ExternalInput")
    bv = nc.dram_tensor("bv", [P, HPC], FP32, kind="ExternalInput")
    owT = nc.dram_tensor("owT", [M, NSL], FP16, kind="ExternalInput")
    obr = nc.dram_tensor("obr", [1, NSL], FP32, kind="ExternalInput")
    cmask = nc.dram_tensor("cmask", [4, P, SC], FP32, kind="ExternalInput")
    ones = nc.dram_tensor("ones", [P, 1], FP32, kind="ExternalInput")
    out = nc.dram_tensor("out", [S, NSL], FP32, kind="ExternalOutput")

    xT_r = xT[:].bitcast(FP32R).rearrange("(mt p) s -> p mt s", p=P)
    wv_r = wv[:].bitcast(FP32R).rearrange("(mt p) n -> p mt n", p=P)

    with tile.TileContext(nc) as tc:
        with (
            tc.tile_pool(name="const", bufs=1) as const,
            tc.tile_pool(name="dram", bufs=1, space="DRAM") as dram,
            tc.tile_pool(name="qkres", bufs=1) as qkres,
        ):
            # constants
            ones_r = const.tile([P, 1], FP32R)
            nc.sync.dma_start(out=ones_r[:], in_=ones[:].bitcast(FP32R))
            bqk_sb = const.tile([P, 8], FP32)
            nc.sync.dma_start(out=bqk_sb[:], in_=bqk[:])
            wsqk_sb = const.tile([P, 8], FP32)
            nc.sync.dma_start(out=wsqk_sb[:], in_=wsqk[:])
            bv_sb = const.tile([P, HPC], FP32)
            nc.sync.dma_start(out=bv_sb[:], in_=bv[:])
            mask_sb = const.tile([P, 4, SC], FP32)
            nc.sync.dma_start(out=mask_sb[:], in_=cmask[:].rearrange("j p q -> p j q"))
            obr_sb = const.tile([1, NSL], FP32)
            nc.sync.dma_start(out=obr_sb[:], in_=obr[:])
            obr_b = const.tile([P, NSL], FP32)
            nc.gpsimd.partition_broadcast(obr_b[:], obr_sb[:])
            wvs_sb = const.tile([1, NSL], FP32)
            nc.sync.dma_start(out=wvs_sb[:], in_=wvs[:])
            wvs_b = const.tile([P, NSL], FP32)
            nc.gpsimd.partition_broadcast(wvs_b[:], wvs_sb[:])
            eps_t = const.tile([1, 1], FP32)
            nc.vector.memset(eps_t[:], EPS)
            owT_sb = const.tile([P, MT, NSL], FP16)
            nc.sync.dma_start(
                out=owT_sb[:], in_=owT[:].rearrange("(it p) j -> p it j", p=P)
            )

            # v, resident in SBUF for the attention phase: [p, st, hpc*D]
            v_sb = qkres.tile([P, ST, NSL], FP32R)
            # q/k staged through DRAM: idx 0..3 = qT per head, 4..7 = kT
            qk_dram = dram.tile([8, P, S], FP32)
            rows_d = dram.tile([NCH, 3, SC], FP32)
            cc_in1 = dram.tile([W1H * P, S], FP16)
            cc_out1 = dram.tile(
                [N_CORES * W1H * P, S], FP16, addr_space="Shared"
            )
            if SPLIT_AG:
                cc_in2a = dram.tile([P, 3 * SC], FP16)
                cc_in2b = dram.tile([P, SC], FP16)
                cc_out2a = dram.tile(
                    [N_CORES * P, 3 * SC], FP16, addr_space="Shared"
                )
                cc_out2b = dram.tile([N_CORES * P, SC], FP16, addr_space="Shared")

            # ---------------- Phase 1: QKV projection (LN folded in) --------
            with (
                tc.tile_pool(name="panel", bufs=2) as panel,
                tc.tile_pool(name="wpool", bufs=2) as wpool,
                tc.tile_pool(name="wvpool", bufs=3) as wvpool,
                tc.tile_pool(name="sqpool", bufs=2) as sqpool,
                tc.tile_pool(name="rows", bufs=2) as rows,
                tc.tile_pool(name="bcast", bufs=2) as bcast,
                tc.tile_pool(name="cols", bufs=2) as colsp,
                tc.tile_pool(name="qkev", bufs=2) as qkev,
                tc.tile_pool(name="psum1", bufs=2, space="PSUM") as psum1,
                tc.tile_pool(name="psumv", bufs=1, space="PSUM") as psumv,
                tc.tile_pool(name="psums", bufs=1, space="PSUM") as psums,
            ):
                for sc in range(NCH):
                    ssl = slice(sc * SC, (sc + 1) * SC)
                    xps = []
                    for mt in range(MT):
                        xp_t = panel.tile(
                            [P, SC], FP32R, tag=f"xp{mt}", name=f"xp{mt}"
                        )
                        nc.sync.dma_start(out=xp_t[:], in_=xT_r[:, mt, ssl])
                        xps.append(xp_t)

                    # column stats over m via ones-matmuls
                    ssum = psums.tile([1, SC], FP32, tag="ssum")
                    ssum2 = psums.tile([1, SC], FP32, tag="ssum2")
                    for mt in range(MT):
                        sq_t = sqpool.tile([P, SC], FP32R, tag="sq")
                        nc.vector.tensor_mul(
                            out=sq_t[:],
                            in0=xps[mt][:].bitcast(FP32),
                            in1=xps[mt][:].bitcast(FP32),
                        )
                        nc.tensor.matmul(
                            ssum[:], ones_r[:], xps[mt][:],
                            start=(mt == 0), stop=(mt == MT - 1),
                        )
                        nc.tensor.matmul(
                            ssum2[:], ones_r[:], sq_t[:],
                            start=(mt == 0), stop=(mt == MT - 1),
                        )

                    mu_row = rows.tile([1, SC], FP32, tag="mu")
                    nc.vector.tensor_scalar_mul(
                        out=mu_row[:], in0=ssum[:], scalar1=1.0 / M
                    )
                    var_row = rows.tile([1, SC], FP32, tag="var")
                    nc.vector.tensor_scalar_mul(
                        out=var_row[:], in0=ssum2[:], scalar1=1.0 / M
                    )
                    std_row = rows.tile([1, SC], FP32, tag="std")
                    nc.vector.tensor_mul(out=std_row[:], in0=mu_row[:], in1=mu_row[:])
                    nc.vector.tensor_sub(out=var_row[:], in0=var_row[:], in1=std_row[:])
                    nc.scalar.activation(
                        out=std_row[:], in_=var_row[:],
                        func=mybir.ActivationFunctionType.Sqrt,
                        bias=eps_t[:],
                    )
                    rstd_row = rows.tile([1, SC], FP32, tag="rstd")
                    nc.vector.reciprocal(out=rstd_row[:], in_=std_row[:])
                    murstd_row = rows.tile([1, SC], FP32, tag="murstd")
                    nc.vector.tensor_mul(
                        out=murstd_row[:], in0=mu_row[:], in1=rstd_row[:]
                    )

                    mu_b = bcast.tile([P, SC], FP32, tag="mub")
                    nc.gpsimd.partition_broadcast(mu_b[:], mu_row[:])
                    rstd_b = bcast.tile([P, SC], FP32, tag="rstdb")
                    nc.gpsimd.partition_broadcast(rstd_b[:], rstd_row[:])

                    # per-s-tile column views of rstd / mu*rstd via DRAM bounce
                    nc.sync.dma_start(out=rows_d[sc, 0:1, :], in_=mu_row[0:1, :])
                    nc.sync.dma_start(out=rows_d[sc, 1:2, :], in_=rstd_row[0:1, :])
                    nc.sync.dma_start(
                        out=rows_d[sc, 2:3, :], in_=murstd_row[0:1, :]
                    )
                    cols_t = colsp.tile([P, 3, SC // P], FP32, tag="cols")
                    nc.sync.dma_start(
                        out=cols_t[:],
                        in_=rows_d[sc].rearrange("k (st p) -> p k st", p=P),
                    )

                    # q/k projections on raw x; LN applied on eviction:
                    #   qk = rstd*(raw - mu*colsum(W)) + bias
                    for nt in range(8):
                        w_t = wpool.tile([P, MT * P], FP32R, tag="w")
                        nc.sync.dma_start(
                            out=w_t[:], in_=wqk_t[nt].bitcast(FP32R)
                        )
                        qkp = psum1.tile([P, SC], FP32, tag="qkp")
                        for mt in range(MT):
                            nc.tensor.matmul(
                                qkp[:],
                                w_t[:, mt * P : (mt + 1) * P],
                                xps[mt][:],
                                start=(mt == 0), stop=(mt == MT - 1),
                            )
                        tmp = qkev.tile([P, SC], FP32, tag="tmp")
                        # wsqk is negated on host: tmp = raw - mu*colsum(W)
                        nc.vector.scalar_tensor_tensor(
                            out=tmp[:],
                            in0=mu_b[:],
                            scalar=wsqk_sb[:, nt : nt + 1],
                            in1=qkp[:],
                            op0=STT_MULT,
                            op1=STT_ADD,
                        )
                        nc.vector.tensor_mul(out=tmp[:], in0=tmp[:], in1=rstd_b[:])
                        qk_ev = qkev.tile([P, SC], FP32R, tag="qkev")
                        nc.vector.tensor_scalar_add(
                            out=qk_ev[:], in0=tmp[:], scalar1=bqk_sb[:, nt : nt + 1]
                        )
                        nc.sync.dma_start(
                            out=qk_dram[nt, :, ssl].bitcast(FP32R), in_=qk_ev[:]
                        )

                    # v projection in natural [s, (h d)] layout, on raw x:
                    #   v = rstd[s]*raw - (mu*rstd)[s]*colsum(Wv)
                    vps = [
                        psumv.tile([P, NSL], FP32, tag=f"vp{st}", name=f"vp{st}")
                        for st in range(SC // P)
                    ]
                    for mt in range(MT):
                        wv_t = wvpool.tile([P, NSL], FP32R, tag="wv")
                        nc.sync.dma_start(
                            out=wv_t[:], in_=wv_r[:, mt, :]
                        )
                        for st in range(SC // P):
                            nc.tensor.matmul(
                                vps[st][:],
                                xps[mt][:, st * P : (st + 1) * P],
                                wv_t[:],
                                start=(mt == 0), stop=(mt == MT - 1),
                            )
                    for st in range(SC // P):
                        vtmp = qkev.tile([P, NSL], FP32, tag="vtmp")
                        nc.vector.tensor_scalar_mul(
                            out=vtmp[:], in0=vps[st][:],
                            scalar1=cols_t[:, 1, st : st + 1],
                        )
                        # wvs negated on host
                        nc.vector.scalar_tensor_tensor(
                            out=v_sb[:, sc * (SC // P) + st, :],
                            in0=wvs_b[:],
                            scalar=cols_t[:, 2, st : st + 1],
                            in1=vtmp[:],
                            op0=STT_MULT,
                            op1=STT_ADD,
                        )

            # ------ Phase 2+3: attention, split AllGather, output proj ------
            with (
                tc.tile_pool(name="ktp", bufs=2) as ktp,
                tc.tile_pool(name="qtp", bufs=2) as qtp,
                tc.tile_pool(name="expp", bufs=4) as expp,
                tc.tile_pool(name="exptmp", bufs=3) as exptmp,
                tc.tile_pool(name="rnorm", bufs=3) as rnorm,
                tc.tile_pool(name="ctxf", bufs=3) as ctxf,
                tc.tile_pool(name="cst", bufs=2) as cstp,
                tc.tile_pool(name="partial", bufs=1) as partp,
                tc.tile_pool(name="outev", bufs=3) as outev,
                tc.tile_pool(name="psst", bufs=2, space="PSUM") as psst,
                tc.tile_pool(name="psctx", bufs=2, space="PSUM") as psctx,
                tc.tile_pool(name="psr", bufs=2, space="PSUM") as psr,
                tc.tile_pool(name="psout", bufs=2, space="PSUM") as psout,
            ):
                for h in range(HPC):
                    for qc in range(NCH):
                        kmax = 4 * (qc + 1)  # causal: k-tiles 0..kmax-1
                        qsl = slice(qc * SC, (qc + 1) * SC)
                        kT_t = ktp.tile([P, S], FP32R, tag="kt")
                        nc.scalar.dma_start(
                            out=kT_t[:, : kmax * P],
                            in_=qk_dram[4 + h, :, : kmax * P].bitcast(FP32R),
                        )
                        qT_t = qtp.tile([P, SC], FP32R, tag="qt")
                        nc.scalar.dma_start(
                            out=qT_t[:], in_=qk_dram[h, :, qsl].bitcast(FP32R)
                        )

                        ctxp = psctx.tile([P, SC], FP32, tag="ctxp")
                        rp = psr.tile([1, SC], FP32, tag="rp")
                        for kt in range(kmax):
                            stp = psst.tile([P, SC], FP32, tag="stp")
                            nc.tensor.matmul(
                                stp[:],
                                kT_t[:, kt * P : (kt + 1) * P],
                                qT_t[:],
                                start=True, stop=True,
                            )
                            expT = expp.tile([P, SC], FP32R, tag="expT")
                            jdiag = kt - 4 * qc
                            if jdiag >= 0:
                                et = exptmp.tile([P, SC], FP32, tag="et")
                                nc.scalar.activation(
                                    out=et[:], in_=stp[:],
                                    func=mybir.ActivationFunctionType.Exp,
                                )
                                nc.vector.tensor_mul(
                                    out=expT[:], in0=et[:], in1=mask_sb[:, jdiag, :]
                                )
                            else:
                                nc.scalar.activation(
                                    out=expT[:], in_=stp[:],
                                    func=mybir.ActivationFunctionType.Exp,
                                )
                            nc.tensor.matmul(
                                ctxp[:],
                                v_sb[:, kt, h * P : (h + 1) * P],
                                expT[:],
                                start=(kt == 0), stop=(kt == kmax - 1),
                            )
                            nc.tensor.matmul(
                                rp[:], ones_r[:], expT[:],
                                start=(kt == 0), stop=(kt == kmax - 1),
                            )

                        rinv = rnorm.tile([1, SC], FP32, tag="rinv")
                        nc.vector.reciprocal(out=rinv[:], in_=rp[:])
                        rinv_b = rnorm.tile([P, SC], FP32, tag="rinvb")
                        nc.gpsimd.partition_broadcast(rinv_b[:], rinv[:])
                        ctx_t = ctxf.tile([P, SC], FP32, tag="ctxt")
                        nc.vector.tensor_mul(out=ctx_t[:], in0=ctxp[:], in1=rinv_b[:])
                        ctx16 = ctxf.tile([P, SC], FP16, tag="ctx16")
                        nc.vector.tensor_scalar_add(
                            out=ctx16[:], in0=ctx_t[:], scalar1=bv_sb[:, h : h + 1]
                        )
                        if h < W1H:
                            nc.gpsimd.dma_start(
                                out=cc_in1[h * P : (h + 1) * P, qsl], in_=ctx16[:]
                            )
                        elif qc < 3:
                            nc.gpsimd.dma_start(
                                out=cc_in2a[:, qc * SC : (qc + 1) * SC],
                                in_=ctx16[:],
                            )
                        else:
                            nc.gpsimd.dma_start(out=cc_in2b[:], in_=ctx16[:])
                        if SPLIT_AG and h == HPC - 1 and qc == 2:
                            nc.gpsimd.collective_compute(
                                "AllGather",
                                mybir.AluOpType.bypass,
                                replica_groups=[list(range(N_CORES))],
                                ins=[cc_in2a.opt()],
                                outs=[cc_out2a.opt()],
                            )

                    if h == W1H - 1:
                        nc.gpsimd.collective_compute(
                            "AllGather",
                            mybir.AluOpType.bypass,
                            replica_groups=[list(range(N_CORES))],
                            ins=[cc_in1.opt()],
                            outs=[cc_out1.opt()],
                        )
                if SPLIT_AG:
                    nc.gpsimd.collective_compute(
                        "AllGather",
                        mybir.AluOpType.bypass,
                        replica_groups=[list(range(N_CORES))],
                        ins=[cc_in2b.opt()],
                        outs=[cc_out2b.opt()],
                    )

                # ---- output projection, two waves over the gathered ctx ----
                # this core's batch half: ranks 4*bh..4*bh+3, bh = rank // 4
                bh = nc.gpsimd.partition_id() // TP
                co1 = cc_out1[:].rearrange(
                    "(b rr h p) s -> p b (rr h) s", b=DP, rr=TP, p=P
                )
                if SPLIT_AG:
                    co2a = cc_out2a[:].rearrange(
                        "(b rr p) s -> p b rr s", b=DP, rr=TP, p=P
                    )
                    co2b = cc_out2b[:].rearrange(
                        "(b rr p) s -> p b rr s", b=DP, rr=TP, p=P
                    )
                partials = []
                for sg in range(ST // 4):
                    sgs = slice(sg * 4 * P, (sg + 1) * 4 * P)
                    cst1 = cstp.tile([P, DP * TP * W1H // DP, 4 * P], FP16, tag="c1")
                    nc.gpsimd.dma_start(
                        out=cst1[:], in_=co1[:, bass.ds(bh, 1), :, sgs]
                    )
                    for stl in range(4):
                        st = sg * 4 + stl
                        op = psout.tile([P, NSL], FP32, tag="op")
                        for ii in range(TP * W1H):
                            rr, hh = divmod(ii, W1H)
                            nc.tensor.matmul(
                                op[:],
                                cst1[:, ii, stl * P : (stl + 1) * P],
                                owT_sb[:, TP * rr + hh, :],
                                start=(ii == 0), stop=(ii == TP * W1H - 1),
                            )
                        if SPLIT_AG:
                            part = partp.tile(
                                [P, NSL], FP32, tag=f"pt{st}", name=f"pt{st}"
                            )
                            nc.vector.tensor_copy(out=part[:], in_=op[:])
                            partials.append(part)
                        else:
                            o_ev = outev.tile([P, NSL], FP32, tag="oev")
                            nc.vector.tensor_add(
                                out=o_ev[:], in0=op[:], in1=obr_b[:]
                            )
                            nc.sync.dma_start(
                                out=out[st * P : (st + 1) * P, :], in_=o_ev[:]
                            )

                for sg in range(ST // 4) if SPLIT_AG else []:
                    cst2 = cstp.tile([P, TP, 4 * P], FP16, tag="c2")
                    if sg < 3:
                        nc.gpsimd.dma_start(
                            out=cst2[:],
                            in_=co2a[
                                :, bass.ds(bh, 1), :,
                                sg * 4 * P : (sg + 1) * 4 * P,
                            ],
                        )
                    else:
                        nc.gpsimd.dma_start(
                            out=cst2[:], in_=co2b[:, bass.ds(bh, 1), :, :]
                        )
                    for stl in range(4):
                        st = sg * 4 + stl
                        op2 = psout.tile([P, NSL], FP32, tag="op")
                        for rr in range(TP):
                            nc.tensor.matmul(
                                op2[:],
                                cst2[:, rr, stl * P : (stl + 1) * P],
                                owT_sb[:, TP * rr + W1H, :],
                                start=(rr == 0), stop=(rr == TP - 1),
                            )
                        o_ev = outev.tile([P, NSL], FP32, tag="oev")
                        nc.vector.tensor_add(
                            out=o_ev[:], in0=op2[:], in1=partials[st][:]
                        )
                        nc.vector.tensor_add(out=o_ev[:], in0=o_ev[:], in1=obr_b[:])
                        nc.sync.dma_start(
                            out=out[st * P : (st + 1) * P, :], in_=o_ev[:]
                        )

    nc.compile()
    return nc


def _prep_inputs(x, ln_g, ln_b, qkvw, qkvb, ow, ob):
    x = np.asarray(x, dtype=np.float32)
    ln_g = np.asarray(ln_g, dtype=np.float32)
    ln_b = np.asarray(ln_b, dtype=np.float32)
    qkvw = np.asarray(qkvw, dtype=np.float32)
    qkvb = np.asarray(qkvb, dtype=np.float32)
    ow = np.asarray(ow, dtype=np.float16)
    ob = np.asarray(ob, dtype=np.float16)

    # fold LayerNorm affine into the QKV weights/bias:
    #   qkv = (xn*g + b) @ W^T + qb = xn @ (W*g)^T + (qb + W @ b)
    qkvwT = np.ascontiguousarray(qkvw.T)  # [M, 3M]
    qkvwT *= ln_g[:, None]
    qkvb_f = qkvb + qkvw @ ln_b

    owT = np.ascontiguousarray(ow.T)  # [M, M] fp16

    kp = np.arange(P)[:, None]
    qf = np.arange(SC)[None, :]
    cmask = np.stack(
        [(qf >= P * j + kp).astype(np.float32) for j in range(4)], axis=0
    )
    ones = np.ones([P, 1], np.float32)

    in_maps = []
    for c in range(N_CORES):
        b, g = divmod(c, TP)
        ns = slice(NSL * g, NSL * (g + 1))
        wqk = np.concatenate([qkvwT[:, ns], qkvwT[:, M:][:, ns]], axis=1)
        # pretile to [nt, p, mt, n] with per-(nt,p) contiguous 8KB runs
        wqk_t = np.ascontiguousarray(
            wqk.reshape(MT, P, 8, P).transpose(2, 1, 0, 3).reshape(8, P, MT * P)
        )
        wv_c = np.ascontiguousarray(qkvwT[:, 2 * M :][:, ns])
        wsqk = np.ascontiguousarray(
            -wqk.sum(axis=0).reshape(8, P).T.astype(np.float32)
        )
        wvs = np.ascontiguousarray(-wv_c.sum(axis=0)[None, :].astype(np.float32))
        bq = qkvb_f[ns].reshape(HPC, P).T
        bk = qkvb_f[M:][ns].reshape(HPC, P).T
        bqk_c = np.ascontiguousarray(np.concatenate([bq, bk], axis=1))
        bv_c = np.ascontiguousarray(qkvb_f[2 * M :][ns].reshape(HPC, P).T)
        in_maps.append(
            {
                "xT": np.ascontiguousarray(x[b].T),
                "wqk_t": wqk_t,
                "wv": wv_c,
                "wsqk": wsqk.astype(np.float32),
                "wvs": wvs,
                "bqk": bqk_c.astype(np.float32),
                "bv": bv_c.astype(np.float32),
                "owT": np.ascontiguousarray(owT[:, ns]),
                "obr": np.ascontiguousarray(
                    ob[ns].astype(np.float32)[None, :]
                ),
                "cmask": cmask,
                "ones": ones,
            }
        )
    return in_maps


def kernel(x, ln_g, ln_b, qkvw, qkvb, ow, ob, _trace=False, _results=None):
    if "nc" not in _cached:
        _cached["nc"] = build_program()
    nc = _cached["nc"]
    in_maps = _prep_inputs(x, ln_g, ln_b, qkvw, qkvb, ow, ob)
    res = run_bass_kernel_spmd(
        nc, in_maps, list(range(N_CORES)), trace=_trace
    )
    if _results is not None:
        _results.append(res)
    full = np.empty([B, S, M], np.float32)
    for c in range(N_CORES):
        b, g = divmod(c, TP)
        full[b, :, NSL * g : NSL * (g + 1)] = res.results[c]["out"]
    return full



# revision 16
# speedup vs baseline: 1.2315x; 1.2315x over previous
"""Megatron-style TP attention kernel for trn2 (8 NeuronCores), v2.

Problem: LayerNorm -> fused QKV -> causal MHA -> fp16 output projection.
  B=2, S=2048, M=2048, H=16 heads, D=128.

Sharding: DP=2 over batch x TP=4 over heads. Core c handles batch c//4 and
heads 4*(c%4)..4*(c%4)+3. Cores AllGather the fp16 context and each core
computes a disjoint 512-column slice of the output projection for its
batch half (emitted transposed, [n, s]; the host transposes back).

v2 changes vs v1:
  - all matmul operands bf16 (tolerance is 2e-2; measured err ~1e-3):
    halves DMA traffic and enables FWL so LDWEIGHTS hides under matmuls.
  - q/k stay resident in SBUF (4MB bf16) -- no DRAM bounce or reload.
  - attention runs qc-outer/head-inner; the ctx AllGather is split into
    wave A (q-chunks 0-2, fires after chunk 2) and wave B (chunk 3), and
    the output projection for chunks 0-2 runs while wave B is in flight.
  - output projection uses ow as the stationary operand and emits
    out^T [n, s] so the gathered ctx [m, s] is the moving operand.

LayerNorm is folded into the QKV eviction: qkv = rstd*(x@W) - murstd*
colsum(W) (+bias), with stats from ones-matmuls on the PE. Softmax needs
no max-subtraction (scores are tiny; masked lanes get exact zeros via
multiplicative bf16 masks after exp); normalization is deferred to the
probs@V eviction.
"""

import numpy as np
import ml_dtypes

import concourse.bass as bass
import concourse.mybir as mybir
import concourse.tile as tile
from concourse import bacc
from concourse.bass_utils import run_bass_kernel_spmd

FP32 = mybir.dt.float32
BF16 = mybir.dt.bfloat16
FP16 = mybir.dt.float16
STT_ADD = mybir.AluOpType.add
STT_MULT = mybir.AluOpType.mult

N_CORES = 8
B, S, M, H = 2, 2048, 2048, 16
D = M // H            # 128
TP = 4                # head groups (tensor parallel)
DP = 2                # batch (data parallel)
HPC = H // TP         # 4 heads per core
NSL = HPC * D         # 512: per-core q/k/v and output column slice
EPS = 1e-5
P = 128
SC = 512              # s-chunk
NCH = S // SC         # 4
MT = M // P           # 16
ST = S // P           # 16
NT = NSL // P         # 4 output-projection column tiles

_cached = {}


def build_program(has_qkv_bias, has_ob):
    nc = bacc.Bacc(
        "TRN2",
        target_bir_lowering=False,
        debug=False,
        num_devices=N_CORES,
        enable_partition_id=True,
    )

    xT = nc.dram_tensor("xT", [M, S], BF16, kind="ExternalInput")
    # q/k weights, host-pretiled: [nt, p, mt*128+n] so each nt-tile DMA is
    # one contiguous run per partition
    wqk_t = nc.dram_tensor("wqk_t", [8, P, MT * P], BF16, kind="ExternalInput")
    wv = nc.dram_tensor("wv", [M, NSL], BF16, kind="ExternalInput")
    # negated column sums of the (g-folded) weights, for the mean fold
    wsqk = nc.dram_tensor("wsqk", [P, 8], FP32, kind="ExternalInput")
    wvs = nc.dram_tensor("wvs", [1, NSL], BF16, kind="ExternalInput")
    bqk = nc.dram_tensor("bqk", [P, 8], FP32, kind="ExternalInput")
    bv = nc.dram_tensor("bv", [P, HPC], FP32, kind="ExternalInput")
    owT = nc.dram_tensor("owT", [M, NSL], FP16, kind="ExternalInput")
    obn = nc.dram_tensor("obn", [P, NT], FP32, kind="ExternalInput")
    cmask = nc.dram_tensor("cmask", [4, P, SC], BF16, kind="ExternalInput")
    ones = nc.dram_tensor("ones", [P, 1], BF16, kind="ExternalInput")
    out = nc.dram_tensor("out", [NSL, S], FP32, kind="ExternalOutput")

    xT_r = xT[:].rearrange("(mt p) s -> p mt s", p=P)
    wv_r = wv[:].rearrange("(mt p) n -> p mt n", p=P)

    with tile.TileContext(nc) as tc:
        with (
            tc.tile_pool(name="const", bufs=1) as const,
            tc.tile_pool(name="dram", bufs=1, space="DRAM") as dram,
            tc.tile_pool(name="qkres", bufs=1) as qkres,
        ):
            # constants (scalar queue; sync queue is kept for x panels)
            ones_b = const.tile([P, 1], BF16)
            nc.scalar.dma_start(out=ones_b[:], in_=ones[:])
            wsqk_sb = const.tile([P, 8], FP32)
            nc.scalar.dma_start(out=wsqk_sb[:], in_=wsqk[:])
            mask_sb = const.tile([P, 4, SC], BF16)
            nc.scalar.dma_start(out=mask_sb[:], in_=cmask[:].rearrange("j p q -> p j q"))
            wvs_sb = const.tile([1, NSL], BF16)
            nc.scalar.dma_start(out=wvs_sb[:], in_=wvs[:])
            wvs_b = const.tile([P, NSL], BF16)
            nc.gpsimd.partition_broadcast(wvs_b[:], wvs_sb[:])
            eps_t = const.tile([1, 1], FP32)
            nc.vector.memset(eps_t[:], EPS)
            # 128-wide ones stationary: the attention rowsum matmul then
            # leaves the softmax denominator broadcast across all PSUM
            # partitions, so no gpsimd partition_broadcast is needed there.
            ones128 = const.tile([P, P], BF16)
            nc.vector.memset(ones128[:], 1.0)
            if has_qkv_bias:
                bqk_sb = const.tile([P, 8], FP32)
                nc.scalar.dma_start(out=bqk_sb[:], in_=bqk[:])
                bv_sb = const.tile([P, HPC], FP32)
                nc.scalar.dma_start(out=bv_sb[:], in_=bv[:])
            if has_ob:
                obn_sb = const.tile([P, NT], FP32)
                nc.scalar.dma_start(out=obn_sb[:], in_=obn[:])
            # needed only in phase 3; scalar queue, after the small consts
            owT_sb = const.tile([P, MT, NSL], FP16)
            nc.scalar.dma_start(
                out=owT_sb[:], in_=owT[:].rearrange("(it p) j -> p it j", p=P)
            )

            # SBUF-resident q/k (idx 0..3 = qT per head, 4..7 = kT) and v
            qk_sb = qkres.tile([P, 8, S], BF16)
            v_sb = qkres.tile([P, ST, NSL], BF16)

            rows_d = dram.tile([NCH, 2, SC], FP32)
            cc_ins = [
                dram.tile([NSL, SC], FP16, name=f"cc_in{qc}")
                for qc in range(NCH)
            ]
            cc_outs = [
                dram.tile(
                    [N_CORES * NSL, SC], FP16, addr_space="Shared",
                    name=f"cc_out{qc}",
                )
                for qc in range(NCH)
            ]

            # ---------------- Phase 1: QKV projection (LN folded in) --------
            with (
                tc.tile_pool(name="panel", bufs=2) as panel,
                tc.tile_pool(name="wpool", bufs=2) as wpool,
                tc.tile_pool(name="wvpool", bufs=3) as wvpool,
                tc.tile_pool(name="sqpool", bufs=2) as sqpool,
                tc.tile_pool(name="rows", bufs=2) as rows,
                tc.tile_pool(name="bcast", bufs=2) as bcast,
                tc.tile_pool(name="cols", bufs=2) as colsp,
                tc.tile_pool(name="qkev", bufs=3) as qkev,
                tc.tile_pool(name="psum1", bufs=2, space="PSUM") as psum1,
                tc.tile_pool(name="psumv", bufs=1, space="PSUM") as psumv,
                tc.tile_pool(name="psums", bufs=1, space="PSUM") as psums,
            ):
                for sc in range(NCH):
                    ssl = slice(sc * SC, (sc + 1) * SC)
                    xps = []
                    for mt in range(MT):
                        xp_t = panel.tile(
                            [P, SC], BF16, tag=f"xp{mt}", name=f"xp{mt}"
                        )
                        nc.sync.dma_start(out=xp_t[:], in_=xT_r[:, mt, ssl])
                        xps.append(xp_t)

                    # column stats over m via ones-matmuls
                    ssum = psums.tile([1, SC], FP32, tag="ssum")
                    ssum2 = psums.tile([1, SC], FP32, tag="ssum2")
                    for mt in range(MT):
                        sq_t = sqpool.tile([P, SC], BF16, tag="sq")
                        nc.scalar.activation(
                            out=sq_t[:], in_=xps[mt][:],
                            func=mybir.ActivationFunctionType.Square,
                        )
                        nc.tensor.matmul(
                            ssum[:], ones_b[:], xps[mt][:],
                            start=(mt == 0), stop=(mt == MT - 1),
                        )
                        nc.tensor.matmul(
                            ssum2[:], ones_b[:], sq_t[:],
                            start=(mt == 0), stop=(mt == MT - 1),
                        )

                    mu_row = rows.tile([1, SC], FP32, tag="mu")
                    nc.vector.tensor_scalar_mul(
                        out=mu_row[:], in0=ssum[:], scalar1=1.0 / M
                    )
                    var_row = rows.tile([1, SC], FP32, tag="var")
                    nc.vector.tensor_scalar_mul(
                        out=var_row[:], in0=ssum2[:], scalar1=1.0 / M
                    )
                    std_row = rows.tile([1, SC], FP32, tag="std")
                    nc.vector.tensor_mul(out=std_row[:], in0=mu_row[:], in1=mu_row[:])
                    nc.vector.tensor_sub(out=var_row[:], in0=var_row[:], in1=std_row[:])
                    nc.scalar.activation(
                        out=std_row[:], in_=var_row[:],
                        func=mybir.ActivationFunctionType.Sqrt,
                        bias=eps_t[:],
                    )
                    rstd_row = rows.tile([1, SC], FP32, tag="rstd")
                    nc.vector.reciprocal(out=rstd_row[:], in_=std_row[:])
                    murstd_row = rows.tile([1, SC], FP32, tag="murstd")
                    nc.vector.tensor_mul(
                        out=murstd_row[:], in0=mu_row[:], in1=rstd_row[:]
                    )
                    murstd_row16 = rows.tile([1, SC], BF16, tag="murstd16")
                    nc.vector.tensor_copy(out=murstd_row16[:], in_=murstd_row[:])

                    rstd_b = bcast.tile([P, SC], FP32, tag="rstdb")
                    nc.gpsimd.partition_broadcast(rstd_b[:], rstd_row[:])
                    murstd_b = bcast.tile([P, SC], BF16, tag="murstdb")
                    nc.gpsimd.partition_broadcast(murstd_b[:], murstd_row16[:])

                    # per-s-tile column views of rstd / mu*rstd via DRAM bounce
                    nc.sync.dma_start(out=rows_d[sc, 0:1, :], in_=rstd_row[0:1, :])
                    nc.sync.dma_start(out=rows_d[sc, 1:2, :], in_=murstd_row[0:1, :])
                    cols_t = colsp.tile([P, 2, SC // P], FP32, tag="cols")
                    nc.sync.dma_start(
                        out=cols_t[:],
                        in_=rows_d[sc].rearrange("k (st p) -> p k st", p=P),
                    )

                    # q/k projections on raw x; LN applied on eviction:
                    #   qk = rstd*raw - murstd*colsum(W) (+ bias)
                    for nt in range(8):
                        w_t = wpool.tile([P, MT * P], BF16, tag="w")
                        nc.sync.dma_start(out=w_t[:], in_=wqk_t[nt])
                        qkp = psum1.tile([P, SC], FP32, tag="qkp")
                        for mt in range(MT):
                            nc.tensor.matmul(
                                qkp[:],
                                w_t[:, mt * P : (mt + 1) * P],
                                xps[mt][:],
                                start=(mt == 0), stop=(mt == MT - 1),
                            )
                        u_t = qkev.tile([P, SC], BF16, tag="u")
                        nc.vector.tensor_mul(out=u_t[:], in0=qkp[:], in1=rstd_b[:])
                        if has_qkv_bias:
                            t2 = qkev.tile([P, SC], BF16, tag="t2")
                            nc.vector.scalar_tensor_tensor(
                                out=t2[:],
                                in0=murstd_b[:],
                                scalar=wsqk_sb[:, nt : nt + 1],
                                in1=u_t[:],
                                op0=STT_MULT,
                                op1=STT_ADD,
                            )
                            nc.vector.tensor_scalar_add(
                                out=qk_sb[:, nt, ssl], in0=t2[:],
                                scalar1=bqk_sb[:, nt : nt + 1],
                            )
                        else:
                            nc.vector.scalar_tensor_tensor(
                                out=qk_sb[:, nt, ssl],
                                in0=murstd_b[:],
                                scalar=wsqk_sb[:, nt : nt + 1],
                                in1=u_t[:],
                                op0=STT_MULT,
                                op1=STT_ADD,
                            )

                    # v projection in natural [s, (h d)] layout, on raw x:
                    #   v = rstd[s]*raw - murstd[s]*colsum(Wv) (+ bias)
                    vps = [
                        psumv.tile([P, NSL], FP32, tag=f"vp{st}", name=f"vp{st}")
                        for st in range(SC // P)
                    ]
                    for mt in range(MT):
                        wv_t = wvpool.tile([P, NSL], BF16, tag="wv")
                        nc.sync.dma_start(out=wv_t[:], in_=wv_r[:, mt, :])
                        for st in range(SC // P):
                            nc.tensor.matmul(
                                vps[st][:],
                                xps[mt][:, st * P : (st + 1) * P],
                                wv_t[:],
                                start=(mt == 0), stop=(mt == MT - 1),
                            )
                    # v bias (if any) is added at the ctx eviction instead
                    # (normalized probs sum to 1, so ctx = ctx_raw + bv[d])
                    for st in range(SC // P):
                        vu = qkev.tile([P, NSL], BF16, tag="vu")
                        nc.vector.tensor_scalar_mul(
                            out=vu[:], in0=vps[st][:],
                            scalar1=cols_t[:, 0, st : st + 1],
                        )
                        nc.vector.scalar_tensor_tensor(
                            out=v_sb[:, sc * (SC // P) + st, :],
                            in0=wvs_b[:],
                            scalar=cols_t[:, 1, st : st + 1],
                            in1=vu[:],
                            op0=STT_MULT,
                            op1=STT_ADD,
                        )

            # ------ Phase 2+3: attention (qc-outer), per-qc AG waves --------
            # The collective_compute instruction occupies the gpsimd queue
            # until the collective completes, so during attention the gpsimd
            # queue carries ONLY the collectives (and the cst loads that wait
            # on them anyway): ctx staging and output stores ride the sync
            # queue, and the softmax-denominator broadcast comes free from the
            # rowsum matmul (128-wide ones stationary -> every PSUM partition
            # holds the rowsum).
            with (
                tc.tile_pool(name="expp", bufs=4) as expp,
                tc.tile_pool(name="exptmp", bufs=3) as exptmp,
                tc.tile_pool(name="rnorm", bufs=3) as rnorm,
                tc.tile_pool(name="ctxf", bufs=4) as ctxf,
                tc.tile_pool(name="cst", bufs=4) as cstp,
                tc.tile_pool(name="outev", bufs=3) as outev,
                tc.tile_pool(name="psst", bufs=3, space="PSUM") as psst,
                tc.tile_pool(name="psctx", bufs=2, space="PSUM") as psctx,
                tc.tile_pool(name="psr", bufs=1, space="PSUM") as psr,
                tc.tile_pool(name="psout", bufs=2, space="PSUM") as psout,
            ):
                # this core's batch half: ranks 4*bh..4*bh+3, bh = rank // 4
                bh = nc.gpsimd.partition_id() // TP
                csts = []
                for qc in range(NCH):
                    kmax = 4 * (qc + 1)  # causal: k-tiles 0..kmax-1
                    qsl = slice(qc * SC, (qc + 1) * SC)
                    for h in range(HPC):
                        ctxp = psctx.tile([P, SC], FP32, tag="ctxp")
                        rp = psr.tile([P, SC], FP32, tag="rp")
                        # software-pipelined: scores(kt+1) is emitted before
                        # ctx/rowsum(kt) so the PE never waits on exp(kt)
                        exps = [None] * kmax

                        def emit_scores(kt):
                            stp = psst.tile([P, SC], FP32, tag="stp")
                            nc.tensor.matmul(
                                stp[:],
                                qk_sb[:, 4 + h, kt * P : (kt + 1) * P],
                                qk_sb[:, h, qsl],
                                start=True, stop=True,
                            )
                            expT = expp.tile([P, SC], BF16, tag="expT")
                            jdiag = kt - 4 * qc
                            if jdiag >= 0:
                                et = exptmp.tile([P, SC], BF16, tag="et")
                                nc.scalar.activation(
                                    out=et[:], in_=stp[:],
                                    func=mybir.ActivationFunctionType.Exp,
                                )
                                nc.vector.tensor_mul(
                                    out=expT[:], in0=et[:], in1=mask_sb[:, jdiag, :]
                                )
                            else:
                                nc.scalar.activation(
                                    out=expT[:], in_=stp[:],
                                    func=mybir.ActivationFunctionType.Exp,
                                )
                            exps[kt] = expT

                        emit_scores(0)
                        for kt in range(kmax):
                            if kt + 1 < kmax:
                                emit_scores(kt + 1)
                            nc.tensor.matmul(
                                ctxp[:],
                                v_sb[:, kt, h * P : (h + 1) * P],
                                exps[kt][:],
                                start=(kt == 0), stop=(kt == kmax - 1),
                            )
                            nc.tensor.matmul(
                                rp[:], ones128[:], exps[kt][:],
                                start=(kt == 0), stop=(kt == kmax - 1),
                            )
                            exps[kt] = None

                        rinv_b = rnorm.tile([P, SC], FP32, tag="rinvb")
                        nc.vector.reciprocal(out=rinv_b[:], in_=rp[:])
                        ctx16 = ctxf.tile([P, SC], FP16, tag="ctx16")
                        nc.vector.tensor_mul(
                            out=ctx16[:], in0=ctxp[:], in1=rinv_b[:]
                        )
                        if has_qkv_bias:
                            nc.vector.tensor_scalar_add(
                                out=ctx16[:], in0=ctx16[:],
                                scalar1=bv_sb[:, h : h + 1],
                            )
                        nc.sync.dma_start(
                            out=cc_ins[qc][h * P : (h + 1) * P, :], in_=ctx16[:]
                        )
                    nc.gpsimd.collective_compute(
                        "AllGather",
                        mybir.AluOpType.bypass,
                        replica_groups=[list(range(N_CORES))],
                        ins=[cc_ins[qc].opt()],
                        outs=[cc_outs[qc].opt()],
                    )
                    co = cc_outs[qc][:].rearrange(
                        "(b rr h p) s -> p b (rr h) s", b=DP, rr=TP, p=P
                    )
                    cst = cstp.tile([P, MT, SC], FP16, tag="cst")
                    nc.gpsimd.dma_start(
                        out=cst[:], in_=co[:, bass.ds(bh, 1), :, :]
                    )
                    csts.append(cst)

                # ---- output projection: out^T[n, s] per q-chunk ----
                for qc in range(NCH):
                    cst = csts[qc]
                    for nt in range(NT):
                        op_ps = psout.tile([P, SC], FP32, tag="op")
                        for mt in range(MT):
                            nc.tensor.matmul(
                                op_ps[:],
                                owT_sb[:, mt, nt * P : (nt + 1) * P],
                                cst[:, mt, :],
                                start=(mt == 0), stop=(mt == MT - 1),
                            )
                        o_ev = outev.tile([P, SC], FP32, tag="oev")
                        if has_ob:
                            nc.vector.tensor_scalar_add(
                                out=o_ev[:], in0=op_ps[:],
                                scalar1=obn_sb[:, nt : nt + 1],
                            )
                        else:
                            nc.vector.tensor_copy(out=o_ev[:], in_=op_ps[:])
                        nc.sync.dma_start(
                            out=out[nt * P : (nt + 1) * P, qc * SC : (qc + 1) * SC],
                            in_=o_ev[:],
                        )

    nc.compile()
    return nc


def _prep_inputs(x, ln_g, ln_b, qkvw, qkvb, ow, ob):
    bf16 = ml_dtypes.bfloat16
    x = np.asarray(x, dtype=np.float32)
    ln_g = np.asarray(ln_g, dtype=np.float32)
    ln_b = np.asarray(ln_b, dtype=np.float32)
    qkvw = np.asarray(qkvw, dtype=np.float32)
    qkvb = np.asarray(qkvb, dtype=np.float32)
    ow = np.asarray(ow, dtype=np.float16)
    ob = np.asarray(ob, dtype=np.float16)

    # fold LayerNorm affine into the QKV weights/bias:
    #   qkv = (xn*g + b) @ W^T + qb = xn @ (W*g)^T + (qb + W @ b)
    qkvwT = np.ascontiguousarray(qkvw.T)  # [M, 3M]
    qkvwT *= ln_g[:, None]
    qkvb_f = qkvb + qkvw @ ln_b

    owT = np.ascontiguousarray(ow.T)  # [M, M] fp16

    kp = np.arange(P)[:, None]
    qf = np.arange(SC)[None, :]
    cmask = np.stack(
        [(qf >= P * j + kp).astype(bf16) for j in range(4)], axis=0
    )
    ones = np.ones([P, 1], bf16)

    has_qkv_bias = bool(np.any(qkvb_f != 0))
    has_ob = bool(np.any(ob != 0))

    in_maps = []
    for c in range(N_CORES):
        b, g = divmod(c, TP)
        ns = slice(NSL * g, NSL * (g + 1))
        wqk = np.concatenate([qkvwT[:, ns], qkvwT[:, M:][:, ns]], axis=1)
        # pretile to [nt, p, mt, n] with per-(nt,p) contiguous runs
        wqk_t = np.ascontiguousarray(
            wqk.reshape(MT, P, 8, P).transpose(2, 1, 0, 3).reshape(8, P, MT * P)
        ).astype(bf16)
        wv_c = np.ascontiguousarray(qkvwT[:, 2 * M :][:, ns])
        wsqk = np.ascontiguousarray(
            -wqk.sum(axis=0).reshape(8, P).T.astype(np.float32)
        )
        wvs = np.ascontiguousarray(-wv_c.sum(axis=0)[None, :]).astype(bf16)
        bq = qkvb_f[ns].reshape(HPC, P).T
        bk = qkvb_f[M:][ns].reshape(HPC, P).T
        bqk_c = np.ascontiguousarray(np.concatenate([bq, bk], axis=1))
        bv_c = np.ascontiguousarray(qkvb_f[2 * M :][ns].reshape(HPC, P).T)
        obn_c = np.ascontiguousarray(
            ob[ns].astype(np.float32).reshape(NT, P).T
        )
        in_maps.append(
            {
                "xT": np.ascontiguousarray(x[b].T).astype(bf16),
                "wqk_t": wqk_t,
                "wv": wv_c.astype(bf16),
                "wsqk": wsqk.astype(np.float32),
                "wvs": wvs,
                "bqk": bqk_c.astype(np.float32),
                "bv": bv_c.astype(np.float32),
                "owT": np.ascontiguousarray(owT[:, ns]),
                "obn": obn_c,
                "cmask": cmask,
                "ones": ones,
            }
        )
    return in_maps, has_qkv_bias, has_ob


def kernel(x, ln_g, ln_b, qkvw, qkvb, ow, ob, _trace=False, _results=None):
    in_maps, has_qkv_bias, has_ob = _prep_inputs(
        x, ln_g, ln_b, qkvw, qkvb, ow, ob
    )
    key = (has_qkv_bias, has_ob)
    if key not in _cached:
        _cached[key] = build_program(has_qkv_bias, has_ob)
    nc = _cached[key]
    res = run_bass_kernel_spmd(
        nc, in_maps, list(range(N_CORES)), trace=_trace
    )
    if _results is not None:
        _results.append(res)
    full = np.empty([B, S, M], np.float32)
    for c in range(N_CORES):
        b, g = divmod(c, TP)
        full[b, :, NSL * g : NSL * (g + 1)] = res.results[c]["out"].T
    return full


# revision 22
# speedup vs baseline: 1.3166x; 1.0691x over previous
"""Megatron-style TP attention kernel for trn2 (8 NeuronCores), v2.

Problem: LayerNorm -> fused QKV -> causal MHA -> fp16 output projection.
  B=2, S=2048, M=2048, H=16 heads, D=128.

Sharding: DP=2 over batch x TP=4 over heads. Core c handles batch c//4 and
heads 4*(c%4)..4*(c%4)+3. Cores AllGather the fp16 context and each core
computes a disjoint 512-column slice of the output projection for its
batch half (emitted transposed, [n, s]; the host transposes back).

v2 changes vs v1:
  - all matmul operands bf16 (tolerance is 2e-2; measured err ~1e-3):
    halves DMA traffic and enables FWL so LDWEIGHTS hides under matmuls.
  - q/k stay resident in SBUF (4MB bf16) -- no DRAM bounce or reload.
  - attention runs qc-outer/head-inner; the ctx AllGather is split into
    wave A (q-chunks 0-2, fires after chunk 2) and wave B (chunk 3), and
    the output projection for chunks 0-2 runs while wave B is in flight.
  - output projection uses ow as the stationary operand and emits
    out^T [n, s] so the gathered ctx [m, s] is the moving operand.

LayerNorm is folded into the QKV eviction: qkv = rstd*(x@W) - murstd*
colsum(W) (+bias), with stats from ones-matmuls on the PE. Softmax needs
no max-subtraction (scores are tiny; masked lanes get exact zeros via
multiplicative bf16 masks after exp); normalization is deferred to the
probs@V eviction.
"""

import numpy as np
import ml_dtypes

import concourse.bass as bass
import concourse.mybir as mybir
import concourse.tile as tile
from concourse import bacc
from concourse.bass_utils import run_bass_kernel_spmd

FP32 = mybir.dt.float32
BF16 = mybir.dt.bfloat16
FP16 = mybir.dt.float16
STT_ADD = mybir.AluOpType.add
STT_MULT = mybir.AluOpType.mult

N_CORES = 8
B, S, M, H = 2, 2048, 2048, 16
D = M // H            # 128
TP = 4                # head groups (tensor parallel)
DP = 2                # batch (data parallel)
HPC = H // TP         # 4 heads per core
NSL = HPC * D         # 512: per-core q/k/v and output column slice
EPS = 1e-5
P = 128
SC = 512              # s-chunk
NCH = S // SC         # 4
MT = M // P           # 16
ST = S // P           # 16
NT = NSL // P         # 4 output-projection column tiles

_cached = {}


def build_program(has_qkv_bias, has_ob):
    nc = bacc.Bacc(
        "TRN2",
        target_bir_lowering=False,
        debug=False,
        num_devices=N_CORES,
        enable_partition_id=True,
    )

    xT = nc.dram_tensor("xT", [M, S], BF16, kind="ExternalInput")
    # q/k weights, host-pretiled: [nt, p, mt*128+n] so each nt-tile DMA is
    # one contiguous run per partition
    wqk_t = nc.dram_tensor("wqk_t", [8, P, MT * P], BF16, kind="ExternalInput")
    wv = nc.dram_tensor("wv", [M, NSL], BF16, kind="ExternalInput")
    # negated column sums of the (g-folded) weights, for the mean fold
    wsqk = nc.dram_tensor("wsqk", [P, 8], FP32, kind="ExternalInput")
    wvs = nc.dram_tensor("wvs", [1, NSL], BF16, kind="ExternalInput")
    bqk = nc.dram_tensor("bqk", [P, 8], FP32, kind="ExternalInput")
    bv = nc.dram_tensor("bv", [P, HPC], FP32, kind="ExternalInput")
    owT = nc.dram_tensor("owT", [M, NSL], FP16, kind="ExternalInput")
    obn = nc.dram_tensor("obn", [P, NT], FP32, kind="ExternalInput")
    cmask = nc.dram_tensor("cmask", [4, P, SC], BF16, kind="ExternalInput")
    ones = nc.dram_tensor("ones", [P, 1], BF16, kind="ExternalInput")
    out = nc.dram_tensor("out", [NSL, S], FP32, kind="ExternalOutput")

    xT_r = xT[:].rearrange("(mt p) s -> p mt s", p=P)
    wv_r = wv[:].rearrange("(mt p) n -> p mt n", p=P)

    with tile.TileContext(nc) as tc:
        with (
            tc.tile_pool(name="const", bufs=1) as const,
            tc.tile_pool(name="dram", bufs=1, space="DRAM") as dram,
            tc.tile_pool(name="qkres", bufs=1) as qkres,
        ):
            # constants (scalar queue; sync queue is kept for x panels)
            ones_b = const.tile([P, 1], BF16)
            nc.scalar.dma_start(out=ones_b[:], in_=ones[:])
            wsqk_sb = const.tile([P, 8], FP32)
            nc.scalar.dma_start(out=wsqk_sb[:], in_=wsqk[:])
            mask_sb = const.tile([P, 4, SC], BF16)
            nc.scalar.dma_start(out=mask_sb[:], in_=cmask[:].rearrange("j p q -> p j q"))
            wvs_sb = const.tile([1, NSL], BF16)
            nc.scalar.dma_start(out=wvs_sb[:], in_=wvs[:])
            wvs_b = const.tile([P, NSL], BF16)
            nc.gpsimd.partition_broadcast(wvs_b[:], wvs_sb[:])
            eps_t = const.tile([1, 1], FP32)
            nc.vector.memset(eps_t[:], EPS)
            # 128-wide ones stationary: the attention rowsum matmul then
            # leaves the softmax denominator broadcast across all PSUM
            # partitions, so no gpsimd partition_broadcast is needed there.
            ones128 = const.tile([P, P], BF16)
            nc.vector.memset(ones128[:], 1.0)
            if has_qkv_bias:
                bqk_sb = const.tile([P, 8], FP32)
                nc.scalar.dma_start(out=bqk_sb[:], in_=bqk[:])
                bv_sb = const.tile([P, HPC], FP32)
                nc.scalar.dma_start(out=bv_sb[:], in_=bv[:])
            if has_ob:
                obn_sb = const.tile([P, NT], FP32)
                nc.scalar.dma_start(out=obn_sb[:], in_=obn[:])
            # needed only in phase 3; scalar queue, after the small consts
            owT_sb = const.tile([P, MT, NSL], FP16)
            nc.scalar.dma_start(
                out=owT_sb[:], in_=owT[:].rearrange("(it p) j -> p it j", p=P)
            )

            # SBUF-resident q/k (idx 0..3 = qT per head, 4..7 = kT) and v
            qk_sb = qkres.tile([P, 8, S], BF16)
            v_sb = qkres.tile([P, ST, NSL], BF16)

            rows_d = dram.tile([NCH, 2, SC], FP32)
            cc_ins = [
                dram.tile([NSL, SC], FP16, name=f"cc_in{qc}")
                for qc in range(NCH)
            ]
            cc_outs = [
                dram.tile(
                    [N_CORES * NSL, SC], FP16, addr_space="Shared",
                    name=f"cc_out{qc}",
                )
                for qc in range(NCH)
            ]

            # ---------------- Phase 1: QKV projection (LN folded in) --------
            with (
                tc.tile_pool(name="panel", bufs=2) as panel,
                tc.tile_pool(name="wpool", bufs=2) as wpool,
                tc.tile_pool(name="wvpool", bufs=3) as wvpool,
                tc.tile_pool(name="sqpool", bufs=2) as sqpool,
                tc.tile_pool(name="rows", bufs=2) as rows,
                tc.tile_pool(name="bcast", bufs=2) as bcast,
                tc.tile_pool(name="cols", bufs=2) as colsp,
                tc.tile_pool(name="qkev", bufs=3) as qkev,
                tc.tile_pool(name="psum1", bufs=2, space="PSUM") as psum1,
                tc.tile_pool(name="psumv", bufs=1, space="PSUM") as psumv,
                tc.tile_pool(name="psums", bufs=1, space="PSUM") as psums,
            ):
                for sc in range(NCH):
                    ssl = slice(sc * SC, (sc + 1) * SC)
                    xpanel = panel.tile([P, MT, SC], BF16, tag="xp")
                    for mt in range(MT):
                        nc.sync.dma_start(
                            out=xpanel[:, mt, :], in_=xT_r[:, mt, ssl]
                        )
                    

                    # column stats over m via ones-matmuls
                    ssum = psums.tile([1, SC], FP32, tag="ssum")
                    ssum2 = psums.tile([1, SC], FP32, tag="ssum2")
                    for mt in range(MT):
                        sq_t = sqpool.tile([P, SC], BF16, tag="sq")
                        nc.vector.tensor_mul(
                            out=sq_t[:], in0=xpanel[:, mt, :], in1=xpanel[:, mt, :]
                        )
                        nc.tensor.matmul(
                            ssum[:], ones_b[:], xpanel[:, mt, :],
                            start=(mt == 0), stop=(mt == MT - 1),
                        )
                        nc.tensor.matmul(
                            ssum2[:], ones_b[:], sq_t[:],
                            start=(mt == 0), stop=(mt == MT - 1),
                        )

                    mu_row = rows.tile([1, SC], FP32, tag="mu")
                    nc.vector.tensor_scalar_mul(
                        out=mu_row[:], in0=ssum[:], scalar1=1.0 / M
                    )
                    var_row = rows.tile([1, SC], FP32, tag="var")
                    nc.vector.tensor_scalar_mul(
                        out=var_row[:], in0=ssum2[:], scalar1=1.0 / M
                    )
                    std_row = rows.tile([1, SC], FP32, tag="std")
                    nc.vector.tensor_mul(out=std_row[:], in0=mu_row[:], in1=mu_row[:])
                    nc.vector.tensor_sub(out=var_row[:], in0=var_row[:], in1=std_row[:])
                    nc.scalar.activation(
                        out=std_row[:], in_=var_row[:],
                        func=mybir.ActivationFunctionType.Sqrt,
                        bias=eps_t[:],
                    )
                    rstd_row = rows.tile([1, SC], FP32, tag="rstd")
                    nc.vector.reciprocal(out=rstd_row[:], in_=std_row[:])
                    murstd_row = rows.tile([1, SC], FP32, tag="murstd")
                    nc.vector.tensor_mul(
                        out=murstd_row[:], in0=mu_row[:], in1=rstd_row[:]
                    )
                    murstd_row16 = rows.tile([1, SC], BF16, tag="murstd16")
                    nc.vector.tensor_copy(out=murstd_row16[:], in_=murstd_row[:])

                    rstd_b = bcast.tile([P, SC], FP32, tag="rstdb")
                    nc.gpsimd.partition_broadcast(rstd_b[:], rstd_row[:])
                    murstd_b = bcast.tile([P, SC], BF16, tag="murstdb")
                    nc.gpsimd.partition_broadcast(murstd_b[:], murstd_row16[:])

                    # per-s-tile column views of rstd / mu*rstd via DRAM bounce
                    nc.sync.dma_start(out=rows_d[sc, 0:1, :], in_=rstd_row[0:1, :])
                    nc.sync.dma_start(out=rows_d[sc, 1:2, :], in_=murstd_row[0:1, :])
                    cols_t = colsp.tile([P, 2, SC // P], FP32, tag="cols")
                    nc.sync.dma_start(
                        out=cols_t[:],
                        in_=rows_d[sc].rearrange("k (st p) -> p k st", p=P),
                    )

                    # q/k projections on raw x; LN applied on eviction:
                    #   qk = rstd*raw - murstd*colsum(W) (+ bias)
                    for nt in range(8):
                        w_t = wpool.tile([P, MT * P], BF16, tag="w")
                        nc.sync.dma_start(out=w_t[:], in_=wqk_t[nt])
                        qkp = psum1.tile([P, SC], FP32, tag="qkp")
                        for mt in range(MT):
                            nc.tensor.matmul(
                                qkp[:],
                                w_t[:, mt * P : (mt + 1) * P],
                                xpanel[:, mt, :],
                                start=(mt == 0), stop=(mt == MT - 1),
                            )
                        u_t = qkev.tile([P, SC], BF16, tag="u")
                        nc.vector.tensor_mul(out=u_t[:], in0=qkp[:], in1=rstd_b[:])
                        if has_qkv_bias:
                            t2 = qkev.tile([P, SC], BF16, tag="t2")
                            nc.vector.scalar_tensor_tensor(
                                out=t2[:],
                                in0=murstd_b[:],
                                scalar=wsqk_sb[:, nt : nt + 1],
                                in1=u_t[:],
                                op0=STT_MULT,
                                op1=STT_ADD,
                            )
                            nc.vector.tensor_scalar_add(
                                out=qk_sb[:, nt, ssl], in0=t2[:],
                                scalar1=bqk_sb[:, nt : nt + 1],
                            )
                        else:
                            nc.vector.scalar_tensor_tensor(
                                out=qk_sb[:, nt, ssl],
                                in0=murstd_b[:],
                                scalar=wsqk_sb[:, nt : nt + 1],
                                in1=u_t[:],
                                op0=STT_MULT,
                                op1=STT_ADD,
                            )

                    # v projection in natural [s, (h d)] layout, on raw x:
                    #   v = rstd[s]*raw - murstd[s]*colsum(Wv) (+ bias)
                    vps = [
                        psumv.tile([P, NSL], FP32, tag=f"vp{st}", name=f"vp{st}")
                        for st in range(SC // P)
                    ]
                    for mt in range(MT):
                        wv_t = wvpool.tile([P, NSL], BF16, tag="wv")
                        nc.sync.dma_start(out=wv_t[:], in_=wv_r[:, mt, :])
                        for st in range(SC // P):
                            nc.tensor.matmul(
                                vps[st][:],
                                xpanel[:, mt, st * P : (st + 1) * P],
                                wv_t[:],
                                start=(mt == 0), stop=(mt == MT - 1),
                            )
                    # v bias (if any) is added at the ctx eviction instead
                    # (normalized probs sum to 1, so ctx = ctx_raw + bv[d])
                    for st in range(SC // P):
                        vu = qkev.tile([P, NSL], BF16, tag="vu")
                        nc.vector.tensor_scalar_mul(
                            out=vu[:], in0=vps[st][:],
                            scalar1=cols_t[:, 0, st : st + 1],
                        )
                        nc.vector.scalar_tensor_tensor(
                            out=v_sb[:, sc * (SC // P) + st, :],
                            in0=wvs_b[:],
                            scalar=cols_t[:, 1, st : st + 1],
                            in1=vu[:],
                            op0=STT_MULT,
                            op1=STT_ADD,
                        )

            # ------ Phase 2+3: attention (qc-outer), per-qc AG waves --------
            # The collective_compute instruction occupies the gpsimd queue
            # until the collective completes, so during attention the gpsimd
            # queue carries ONLY the collectives (and the cst loads that wait
            # on them anyway): ctx staging and output stores ride the sync
            # queue, and the softmax-denominator broadcast comes free from the
            # rowsum matmul (128-wide ones stationary -> every PSUM partition
            # holds the rowsum).
            with (
                tc.tile_pool(name="expp", bufs=4) as expp,
                tc.tile_pool(name="exptmp", bufs=3) as exptmp,
                tc.tile_pool(name="rnorm", bufs=3) as rnorm,
                tc.tile_pool(name="ctxf", bufs=4) as ctxf,
                tc.tile_pool(name="cst", bufs=4) as cstp,
                tc.tile_pool(name="outev", bufs=3) as outev,
                tc.tile_pool(name="psst", bufs=3, space="PSUM") as psst,
                tc.tile_pool(name="psctx", bufs=2, space="PSUM") as psctx,
                tc.tile_pool(name="psr", bufs=1, space="PSUM") as psr,
                tc.tile_pool(name="psout", bufs=2, space="PSUM") as psout,
            ):
                # this core's batch half: ranks 4*bh..4*bh+3, bh = rank // 4
                bh = nc.gpsimd.partition_id() // TP
                # q-chunks run in REVERSE order: the last AllGather wave is
                # then the smallest chunk (qc0) and its flight hides behind
                # the other three chunks' output projections.
                QC_ORDER = [2, 3, 1, 0]
                # one flat software-pipelined PE stream over (qc, h, kt):
                # scores run LOOKAHEAD tiles ahead of the dependent ctx /
                # rowsum matmuls so the PE never waits on exp.
                steps = [
                    (qc, h, kt)
                    for qc in QC_ORDER
                    for h in range(HPC)
                    for kt in range(4 * (qc + 1))
                ]
                LOOKAHEAD = 2
                exps = {}
                csts = {}
                cur = {}  # live psum tiles for the head being consumed

                def emit_scores(i):
                    qc, h, kt = steps[i]
                    stp = psst.tile([P, SC], FP32, tag="stp")
                    nc.tensor.matmul(
                        stp[:],
                        qk_sb[:, 4 + h, kt * P : (kt + 1) * P],
                        qk_sb[:, h, qc * SC : (qc + 1) * SC],
                        start=True, stop=True,
                    )
                    expT = expp.tile([P, SC], BF16, tag="expT")
                    jdiag = kt - 4 * qc
                    if jdiag >= 0:
                        et = exptmp.tile([P, SC], BF16, tag="et")
                        nc.scalar.activation(
                            out=et[:], in_=stp[:],
                            func=mybir.ActivationFunctionType.Exp,
                        )
                        nc.vector.tensor_mul(
                            out=expT[:], in0=et[:], in1=mask_sb[:, jdiag, :]
                        )
                    else:
                        nc.scalar.activation(
                            out=expT[:], in_=stp[:],
                            func=mybir.ActivationFunctionType.Exp,
                        )
                    exps[i] = expT

                def consume(i):
                    qc, h, kt = steps[i]
                    kmax = 4 * (qc + 1)
                    if kt == 0:
                        cur["ctxp"] = psctx.tile([P, SC], FP32, tag="ctxp", name="ctxp")
                        cur["rp"] = psr.tile([P, SC], FP32, tag="rp", name="rp")
                    nc.tensor.matmul(
                        cur["ctxp"][:],
                        v_sb[:, kt, h * P : (h + 1) * P],
                        exps[i][:],
                        start=(kt == 0), stop=(kt == kmax - 1),
                    )
                    nc.tensor.matmul(
                        cur["rp"][:], ones128[:], exps[i][:],
                        start=(kt == 0), stop=(kt == kmax - 1),
                    )
                    del exps[i]
                    if kt != kmax - 1:
                        return
                    # head (qc, h) done: normalize, stage, and on the last
                    # head of the chunk fire its AllGather + gathered load
                    rinv_b = rnorm.tile([P, SC], FP32, tag="rinvb")
                    nc.vector.reciprocal_approx_fast(
                        out=rinv_b[:], in_=cur["rp"][:]
                    )
                    ctx16 = ctxf.tile([P, SC], FP16, tag="ctx16")
                    nc.vector.tensor_mul(
                        out=ctx16[:], in0=cur["ctxp"][:], in1=rinv_b[:]
                    )
                    if has_qkv_bias:
                        nc.vector.tensor_scalar_add(
                            out=ctx16[:], in0=ctx16[:],
                            scalar1=bv_sb[:, h : h + 1],
                        )
                    nc.sync.dma_start(
                        out=cc_ins[qc][h * P : (h + 1) * P, :], in_=ctx16[:]
                    )
                    if h == HPC - 1:
                        nc.gpsimd.collective_compute(
                            "AllGather",
                            mybir.AluOpType.bypass,
                            replica_groups=[list(range(N_CORES))],
                            ins=[cc_ins[qc].opt()],
                            outs=[cc_outs[qc].opt()],
                        )
                        co = cc_outs[qc][:].rearrange(
                            "(b rr h p) s -> p b (rr h) s", b=DP, rr=TP, p=P
                        )
                        cst = cstp.tile([P, MT, SC], FP16, tag="cst")
                        nc.gpsimd.dma_start(
                            out=cst[:], in_=co[:, bass.ds(bh, 1), :, :]
                        )
                        csts[qc] = cst

                n_steps = len(steps)
                for i in range(n_steps):
                    emit_scores(i)
                    if i >= LOOKAHEAD:
                        consume(i - LOOKAHEAD)
                for i in range(n_steps - LOOKAHEAD, n_steps):
                    consume(i)

                # ---- output projection: out^T[n, s] per q-chunk ----
                for qc in QC_ORDER:
                    cst = csts[qc]
                    for nt in range(NT):
                        op_ps = psout.tile([P, SC], FP32, tag="op")
                        for mt in range(MT):
                            nc.tensor.matmul(
                                op_ps[:],
                                owT_sb[:, mt, nt * P : (nt + 1) * P],
                                cst[:, mt, :],
                                start=(mt == 0), stop=(mt == MT - 1),
                            )
                        o_ev = outev.tile([P, SC], FP32, tag="oev")
                        if has_ob:
                            nc.vector.tensor_scalar_add(
                                out=o_ev[:], in0=op_ps[:],
                                scalar1=obn_sb[:, nt : nt + 1],
                            )
                        else:
                            nc.vector.tensor_copy(out=o_ev[:], in_=op_ps[:])
                        nc.sync.dma_start(
                            out=out[nt * P : (nt + 1) * P, qc * SC : (qc + 1) * SC],
                            in_=o_ev[:],
                        )

    nc.compile()
    return nc


def _prep_inputs(x, ln_g, ln_b, qkvw, qkvb, ow, ob):
    bf16 = ml_dtypes.bfloat16
    x = np.asarray(x, dtype=np.float32)
    ln_g = np.asarray(ln_g, dtype=np.float32)
    ln_b = np.asarray(ln_b, dtype=np.float32)
    qkvw = np.asarray(qkvw, dtype=np.float32)
    qkvb = np.asarray(qkvb, dtype=np.float32)
    ow = np.asarray(ow, dtype=np.float16)
    ob = np.asarray(ob, dtype=np.float16)

    # fold LayerNorm affine into the QKV weights/bias:
    #   qkv = (xn*g + b) @ W^T + qb = xn @ (W*g)^T + (qb + W @ b)
    qkvwT = np.ascontiguousarray(qkvw.T)  # [M, 3M]
    qkvwT *= ln_g[:, None]
    qkvb_f = qkvb + qkvw @ ln_b

    owT = np.ascontiguousarray(ow.T)  # [M, M] fp16

    kp = np.arange(P)[:, None]
    qf = np.arange(SC)[None, :]
    cmask = np.stack(
        [(qf >= P * j + kp).astype(bf16) for j in range(4)], axis=0
    )
    ones = np.ones([P, 1], bf16)

    has_qkv_bias = bool(np.any(qkvb_f != 0))
    has_ob = bool(np.any(ob != 0))

    in_maps = []
    for c in range(N_CORES):
        b, g = divmod(c, TP)
        ns = slice(NSL * g, NSL * (g + 1))
        wqk = np.concatenate([qkvwT[:, ns], qkvwT[:, M:][:, ns]], axis=1)
        # pretile to [nt, p, mt, n] with per-(nt,p) contiguous runs
        wqk_t = np.ascontiguousarray(
            wqk.reshape(MT, P, 8, P).transpose(2, 1, 0, 3).reshape(8, P, MT * P)
        ).astype(bf16)
        wv_c = np.ascontiguousarray(qkvwT[:, 2 * M :][:, ns])
        wsqk = np.ascontiguousarray(
            -wqk.sum(axis=0).reshape(8, P).T.astype(np.float32)
        )
        wvs = np.ascontiguousarray(-wv_c.sum(axis=0)[None, :]).astype(bf16)
        bq = qkvb_f[ns].reshape(HPC, P).T
        bk = qkvb_f[M:][ns].reshape(HPC, P).T
        bqk_c = np.ascontiguousarray(np.concatenate([bq, bk], axis=1))
        bv_c = np.ascontiguousarray(qkvb_f[2 * M :][ns].reshape(HPC, P).T)
        obn_c = np.ascontiguousarray(
            ob[ns].astype(np.float32).reshape(NT, P).T
        )
        in_maps.append(
            {
                "xT": np.ascontiguousarray(x[b].T).astype(bf16),
                "wqk_t": wqk_t,
                "wv": wv_c.astype(bf16),
                "wsqk": wsqk.astype(np.float32),
                "wvs": wvs,
                "bqk": bqk_c.astype(np.float32),
                "bv": bv_c.astype(np.float32),
                "owT": np.ascontiguousarray(owT[:, ns]),
                "obn": obn_c,
                "cmask": cmask,
                "ones": ones,
            }
        )
    return in_maps, has_qkv_bias, has_ob


def kernel(x, ln_g, ln_b, qkvw, qkvb, ow, ob, _trace=False, _results=None):
    in_maps, has_qkv_bias, has_ob = _prep_inputs(
        x, ln_g, ln_b, qkvw, qkvb, ow, ob
    )
    key = (has_qkv_bias, has_ob)
    if key not in _cached:
        _cached[key] = build_program(has_qkv_bias, has_ob)
    nc = _cached[key]
    res = run_bass_kernel_spmd(
        nc, in_maps, list(range(N_CORES)), trace=_trace
    )
    if _results is not None:
        _results.append(res)
    full = np.empty([B, S, M], np.float32)
    for c in range(N_CORES):
        b, g = divmod(c, TP)
        full[b, :, NSL * g : NSL * (g + 1)] = res.results[c]["out"].T
    return full
